# revision 6
# baseline (speedup 1.0000x reference)
"""Trainium2 Bass kernel for nn_Ballistics: per-batch attack/release one-pole
envelope follower y[t] = (1-c)*y[t-1] + c*x[t], c = at if x[t] > y[t-1] else rt.

Algorithm (per core, 8 batch rows):
  Work in the margin variable u[j] = y[j-1] - x[j]:
      u[j+1] = a_j * u[j] - dx[j+1],   dx[j] = x[j] - x[j-1],
      a_j = a_rt if u[j] >= 0 else a_at        (a_* = 1 - coeff)
      y[j] = x[j+1] + u[j+1]
  The branch is sign(u): solve by predicate iteration — freeze signs from the
  previous iterate (ScalarE Sign + per-partition affine), solve the then-linear
  recurrence exactly with the native tensor_tensor_scan (VectorE), repeat K
  times.  The map contracts (|a|<1), so the time axis is chunked into 64
  chunks x 4096 steps with a 256-step warm-up whose output is discarded;
  chunk 0 is padded with x=1.0 so y[-1]=1 exactly.
  128 partitions = 8 rows x 16 chunks; 4 sequential groups cover 64 chunks.
K=6 verified to reach the exact fixed point (global rel err ~3e-7).
"""
import sys
for p in ("/opt/trn_rl_repo", "/root/.axon_site/_ro/trn_rl_repo"):
    if p not in sys.path:
        sys.path.insert(0, p)

import numpy as np

B, T = 64, 262144
NCORES = 8
RPC = B // NCORES          # rows per core
L = 4096                   # chunk length (output steps per chunk)
W = 256                    # warm-up steps (discarded)
K = 6                      # predicate iterations
C = T // L                 # chunks per row (64)
QP = 128 // RPC            # chunks in flight per row (16)
G = C // QP                # sequential groups (4)
N = L + W + 1              # scan steps per window
NW = N + 1                 # x-window columns per partition

_cache = {}


def _build(reps=1):
    import concourse.bacc as bacc
    import concourse.mybir as mybir
    import concourse.tile as tile
    import concourse.bass as bass

    f32 = mybir.dt.float32
    Alu = mybir.AluOpType
    Act = mybir.ActivationFunctionType

    nc = bacc.Bacc("TRN2", target_bir_lowering=False, debug=False,
                   num_devices=NCORES)
    x_d = nc.dram_tensor("x", [RPC, T], f32, kind="ExternalInput")
    mid_d = nc.dram_tensor("mid", [128, 1], f32, kind="ExternalInput")
    hdl_d = nc.dram_tensor("hdl", [128, 1], f32, kind="ExternalInput")
    y_d = nc.dram_tensor("y", [RPC, T], f32, kind="ExternalOutput")

    with tile.TileContext(nc) as tc:
        with tc.tile_pool(name="cpool", bufs=1) as cpool, \
             tc.tile_pool(name="xpool", bufs=2) as xpool, \
             tc.tile_pool(name="ypool", bufs=2) as ypool, \
             tc.tile_pool(name="wpool", bufs=2) as wpool:
            mid_s = cpool.tile([128, 1], f32, tag="mid")
            hdl_s = cpool.tile([128, 1], f32, tag="hdl")
            nc.sync.dma_start(mid_s[:, :], mid_d.ap()[:, :])
            nc.sync.dma_start(hdl_s[:, :], hdl_d.ap()[:, :])

            for _rep in range(reps):
                for gr in range(G):
                    xt = xpool.tile([128, NW], f32, tag="xt")
                    # ---- gather DMA: xt[r*16+q, k] = x[r, (gr*16+q)*L - W - 1 + k]
                    base = gr * QP * L - W - 1
                    if gr == 0:
                        # chunk 0 (q=0): pad cols [0, W+1) with 1.0, real x[0:L+1].
                        # Memset pad on all partitions; DMAs overwrite q>=1.
                        nc.vector.memset(xt[:, 0:W + 1], 1.0)
                        for r in range(RPC):
                            p0 = r * QP
                            nc.sync.dma_start(
                                xt[p0:p0 + 1, W + 1:NW],
                                bass.AP(x_d, r * T, [[1, L + 1]]))
                            nc.sync.dma_start(
                                xt[p0 + 1:p0 + QP, :],
                                bass.AP(x_d, r * T + L - W - 1,
                                        [[L, QP - 1], [1, NW]]))
                    elif gr == G - 1:
                        # last chunk (q=15): col NW-1 would be x[T] -> pad 0.0
                        nc.vector.memset(xt[:, NW - 1:NW], 0.0)
                        for r in range(RPC):
                            p0 = r * QP
                            nc.sync.dma_start(
                                xt[p0:p0 + QP - 1, :],
                                bass.AP(x_d, r * T + base, [[L, QP - 1], [1, NW]]))
                            nc.sync.dma_start(
                                xt[p0 + QP - 1:p0 + QP, 0:NW - 1],
                                bass.AP(x_d, r * T + base + (QP - 1) * L,
                                        [[1, NW - 1]]))
                    else:
                        for r in range(RPC):
                            nc.sync.dma_start(
                                xt[r * QP:(r + 1) * QP, :],
                                bass.AP(x_d, r * T + base, [[L, QP], [1, NW]]))

                    # ---- dx[m] = xt[m+1] - xt[m]
                    dxt = wpool.tile([128, N], f32, tag="dx")
                    nc.vector.tensor_tensor(dxt[:, :], xt[:, 1:NW], xt[:, 0:N],
                                            Alu.subtract)

                    # u: col 0 is the (absorbed) initial state, kept at 0
                    ut = wpool.tile([128, NW], f32, tag="ut")
                    att = wpool.tile([128, NW], f32, tag="att")
                    nc.gpsimd.memset(ut[:, 0:1], 0.0)
                    nc.gpsimd.memset(att[:, 0:1], 0.0)

                    for k in range(K):
                        # sign pass (ScalarE): att[m] = sign(u[m]) {-1,0,1}
                        if k == 0:
                            # seed u0[m] = -dx[m] (y ~= x): shifted read of dx
                            nc.scalar.activation(att[:, 1:NW], dxt[:, 0:N],
                                                 Act.Sign, scale=-1.0)
                        else:
                            nc.scalar.activation(att[:, :], ut[:, :], Act.Sign)
                        # affine (ScalarE, in place): a = sign*hdl + mid
                        nc.scalar.activation(att[:, :], att[:, :], Act.Identity,
                                             bias=mid_s[:, 0:1],
                                             scale=hdl_s[:, 0:1])
                        # exact linear scan with frozen signs (VectorE)
                        nc.vector.tensor_tensor_scan(
                            ut[:, 1:NW], att[:, 0:N], dxt[:, :], 0.0,
                            Alu.mult, Alu.subtract)

                    # ---- y[l] = xt[W+2+l] + u[W+2+l],  l in [0, L)
                    yt = ypool.tile([128, L], f32, tag="yt")
                    nc.vector.tensor_tensor(
                        yt[:, :], xt[:, W + 2:W + 2 + L],
                        ut[:, W + 2:W + 2 + L], Alu.add)
                    nc.sync.dma_start(
                        bass.AP(y_d, gr * QP * L, [[T, RPC], [L, QP], [1, L]]),
                        yt[:, :])

    nc.compile()
    return nc


def _get_nc(reps=1):
    if reps not in _cache:
        _cache[reps] = _build(reps)
    return _cache[reps]


def _coeffs(z_alpha):
    z = np.asarray(z_alpha, dtype=np.float32)
    ts = (np.float32(1.0) / (np.float32(1.0) + np.exp(-z, dtype=np.float32)))
    at = ts[:, 0].astype(np.float32)
    rt = ts[:, 1].astype(np.float32)
    a_at = (np.float32(1.0) - at).astype(np.float32)
    a_rt = (np.float32(1.0) - rt).astype(np.float32)
    return a_at, a_rt


def kernel(signal, z_alpha):
    from concourse import bass_utils
    signal = np.ascontiguousarray(np.asarray(signal, dtype=np.float32))
    a_at, a_rt = _coeffs(z_alpha)
    mid = ((a_at + a_rt) * np.float32(0.5)).astype(np.float32)
    hdl = ((a_rt - a_at) * np.float32(0.5)).astype(np.float32)

    nc = _get_nc()
    in_maps = []
    prow = np.arange(128) // QP  # local row of each partition
    for ci in range(NCORES):
        rows = slice(ci * RPC, (ci + 1) * RPC)
        in_maps.append({
            "x": signal[rows],
            "mid": mid[ci * RPC + prow][:, None].astype(np.float32),
            "hdl": hdl[ci * RPC + prow][:, None].astype(np.float32),
        })
    res = bass_utils.run_bass_kernel_spmd(nc, in_maps, core_ids=list(range(NCORES)))
    out = np.concatenate([r["y"] for r in res.results], axis=0)
    return out.astype(np.float32)


if __name__ == "__main__":
    rng = np.random.default_rng(0)
    sig = rng.standard_normal((B, T)).astype(np.float32)
    za = rng.standard_normal((B, 2)).astype(np.float32)
    y = kernel(sig, za)
    print("kernel ran:", y.shape, y.dtype)
